# revision 25
# baseline (speedup 1.0000x reference)
"""GraphSAGE 2-layer encoder on 8 Trainium2 NeuronCores.

Reference computation (PyG SAGEConv, aggr='mean', 2 layers, leaky-relu 0.5):
    h = x
    for layer in (0, 1):
        mean_i = (1/max(deg_i,1)) * sum_{j in N(i)} h_j
        h = leaky( mean @ Wl + h @ Wr + bl )
    return (h, x)

Strategy: shard the 50000 dst nodes across 8 cores (6250 each). Host sorts
each core's nodes by in-degree (round-robin by global degree rank, so every
core's tile t covers the same degree band) and assigns every edge a
(tile, slot, partition) so a message tile [128, Kt*256] is node-aligned:
slot (p, k) holds the src features of node p's k-th in-edge.

On-device random gathers bottleneck on Q7 software descriptor generation
(~9.3 ns per 512B row descriptor -> ~1 ms/layer/core for 100K edges), so the
host performs the slot gather between launches (allowed by the full-inputs
contract -- the host already re-shards h between the two launches) and the
device streams the pre-gathered message array with large affine DMAs.
Per tile: one contiguous DMA, a PSUM-accumulating identity-matmul chain for
the segment sum, deg-reciprocal scale, PE transpose, and the two GEMMs +
bias + leaky-relu. Features/weights are bf16 (1 PE cycle/row vs 4 for
fp32); accumulation stays fp32 in PSUM.

Each layer is one SPMD bass launch; the h exchange between layers goes
through the host.
"""

import numpy as np
from contextlib import ExitStack

import ml_dtypes

import concourse.bass as bass
import concourse.bacc as bacc
import concourse.mybir as mybir
import concourse.tile as tile
from concourse.bass_utils import run_bass_kernel_spmd
from concourse.masks import make_identity

P = 128
N_NODES = 50000
DIM = 256
N_CORES = 8

F32 = mybir.dt.float32
BF16 = mybir.dt.bfloat16
BF = ml_dtypes.bfloat16


# ---------------------------------------------------------------- host prep
def _prep_graph(edge_index, n_nodes, n_cores):
    """Slot assignment: returns per-core slot grid [P, C_total] of global
    node ids (pad -> n_nodes, the zero row), recip [P, T], node_order,
    K_list (chunk count per tile, shared by all cores)."""
    src = np.asarray(edge_index[0], dtype=np.int64)
    dst = np.asarray(edge_index[1], dtype=np.int64)
    deg = np.bincount(dst, minlength=n_nodes)

    order = np.argsort(dst, kind="stable")
    srcs_sorted = src[order].astype(np.int64)
    cum = np.zeros(n_nodes + 1, dtype=np.int64)
    np.cumsum(deg, out=cum[1:])

    nsh = n_nodes // n_cores
    T = (nsh + P - 1) // P
    nsh_pad = T * P

    # node -> core by global degree rank, round-robin: tile t then holds the
    # same degree band on every core, so the shared per-tile chunk count
    # K_t = max-degree-in-tile has no cross-core slack
    node_order = np.full((n_cores, nsh_pad), -1, dtype=np.int64)
    deg_slot = np.zeros((n_cores, nsh_pad), dtype=np.int64)
    rank = np.argsort(-deg, kind="stable")
    for c in range(n_cores):
        g = rank[c::n_cores][:nsh]
        node_order[c, :nsh] = g
        deg_slot[c, :nsh] = deg[g]

    K_list = []
    for t in range(T):
        K_t = int(deg_slot[:, t * P : (t + 1) * P].max())
        K_list.append(max(K_t, 1))
    C_total = int(np.sum(K_list))
    col_off = np.concatenate([[0], np.cumsum(K_list)]).astype(np.int64)

    slots = np.full((n_cores, P, C_total), n_nodes, dtype=np.int64)
    recip_arr = np.zeros((n_cores, P, T), dtype=np.float32)
    for c in range(n_cores):
        for t in range(T):
            Kt = K_list[t]
            nodes = node_order[c, t * P : (t + 1) * P]
            degs = deg_slot[c, t * P : (t + 1) * P]
            recip_arr[c, :, t] = 1.0 / np.maximum(degs, 1)
            for p in range(P):
                nd = nodes[p]
                if nd < 0:
                    continue
                d = int(degs[p])
                if d:
                    slots[c, p, col_off[t] : col_off[t] + d] = srcs_sorted[
                        cum[nd] : cum[nd] + d
                    ]

    # pair adjacent tiles (similar degree bands): the agg matmul then runs
    # with 512-wide free dim, halving instruction count. msg layout per pair
    # j (tiles a=2j, b=2j+1): chunk k = [tile-a chunk k | tile-b chunk k].
    npairs = T // 2
    K_pair = [max(K_list[2 * j], K_list[2 * j + 1]) for j in range(npairs)]
    C2 = 2 * int(np.sum(K_pair)) + (K_list[-1] if T % 2 else 0)
    slots2 = np.full((n_cores, P, C2), n_nodes, dtype=np.int64)
    cur = 0
    for j in range(npairs):
        a, b = 2 * j, 2 * j + 1
        for k in range(K_pair[j]):
            if k < K_list[a]:
                slots2[:, :, cur] = slots[:, :, col_off[a] + k]
            cur += 1
            if k < K_list[b]:
                slots2[:, :, cur] = slots[:, :, col_off[b] + k]
            cur += 1
    if T % 2:
        t = T - 1
        Kt = K_list[t]
        slots2[:, :, cur : cur + Kt] = slots[:, :, col_off[t] : col_off[t] + Kt]
        cur += Kt
    assert cur == C2

    return dict(
        slots=slots2,
        recip=recip_arr,
        node_order=node_order,
        K_list=K_list,
        K_pair=K_pair,
        C2=C2,
        T=T,
        nsh=nsh,
        nsh_pad=nsh_pad,
        C_total=C_total,
    )


# ------------------------------------------------------------ device program
def build_layer_nc(K_list, nsh_pad, dim=DIM, n_cores=N_CORES, t_limit=None):
    """One SAGEConv layer (mean-aggregate + linear + leaky 0.5) over a
    host-pre-gathered, pair-interleaved message array."""
    T = len(K_list)
    npairs = T // 2
    K_pair = [max(K_list[2 * j], K_list[2 * j + 1]) for j in range(npairs)]
    C2 = 2 * int(np.sum(K_pair)) + (K_list[-1] if T % 2 else 0)
    Kp_max = int(np.max(K_pair))
    assert dim % P == 0
    KC = dim // P

    nc = bacc.Bacc(
        "TRN2",
        target_bir_lowering=False,
        debug=False,
        enable_asserts=False,
        num_devices=n_cores,
    )
    msg = nc.dram_tensor("msg", [P, C2 * dim], BF16, kind="ExternalInput").ap()
    featT = nc.dram_tensor("featT", [dim, nsh_pad], BF16, kind="ExternalInput").ap()
    recip = nc.dram_tensor("recip", [P, T], F32, kind="ExternalInput").ap()
    wl = nc.dram_tensor("wl", [dim, dim], BF16, kind="ExternalInput").ap()
    wr = nc.dram_tensor("wr", [dim, dim], BF16, kind="ExternalInput").ap()
    bl = nc.dram_tensor("bl", [1, dim], BF16, kind="ExternalInput").ap()
    hout = nc.dram_tensor("hout", [nsh_pad, dim], F32, kind="ExternalOutput").ap()

    with tile.TileContext(nc) as tc, ExitStack() as ctx:
        const = ctx.enter_context(tc.tile_pool(name="const", bufs=1))
        work = ctx.enter_context(tc.tile_pool(name="work", bufs=3))
        psum = ctx.enter_context(tc.tile_pool(name="psum", bufs=2, space="PSUM"))

        ident = const.tile([P, P], BF16)
        make_identity(nc, ident[:])
        ident32 = const.tile([P, P], F32)
        make_identity(nc, ident32[:])
        ones_row = const.tile([1, P], BF16)
        nc.gpsimd.memset(ones_row[:], 1.0)

        recip_sb = const.tile([P, T], F32)
        nc.sync.dma_start(out=recip_sb[:], in_=recip[:, :])
        bias_sb = const.tile([1, dim], BF16)
        nc.sync.dma_start(out=bias_sb[:], in_=bl[:, :])

        wl_sb = const.tile([P, KC * dim], BF16)
        wr_sb = const.tile([P, KC * dim], BF16)
        for kc in range(KC):
            nc.sync.dma_start(
                out=wl_sb[:, kc * dim : (kc + 1) * dim],
                in_=wl[kc * P : (kc + 1) * P, :],
            )
            nc.sync.dma_start(
                out=wr_sb[:, kc * dim : (kc + 1) * dim],
                in_=wr[kc * P : (kc + 1) * P, :],
            )

        def tile_tail(t, mean_half):
            """transpose + GEMMs + bias + leaky + store for one tile, given
            its fp32 mean [P, dim] slice."""
            featT_t = work.tile([P, KC * P], BF16, tag="featT")
            for kc in range(KC):
                nc.sync.dma_start(
                    out=featT_t[:, kc * P : (kc + 1) * P],
                    in_=featT[kc * P : (kc + 1) * P, t * P : (t + 1) * P],
                )
            meanT_sb = work.tile([P, KC * P], BF16, tag="meanT")
            for kc in range(KC):
                p_tr = psum.tile([P, P], F32, tag="tr")
                nc.tensor.transpose(
                    out=p_tr[:],
                    in_=mean_half[:, kc * P : (kc + 1) * P],
                    identity=ident32[:],
                )
                nc.vector.tensor_copy(
                    out=meanT_sb[:, kc * P : (kc + 1) * P], in_=p_tr[:]
                )
            p_out = psum.tile([P, dim], F32, tag="out")
            for kc in range(KC):
                nc.tensor.matmul(
                    out=p_out[:],
                    lhsT=meanT_sb[:, kc * P : (kc + 1) * P],
                    rhs=wl_sb[:, kc * dim : (kc + 1) * dim],
                    start=(kc == 0),
                    stop=False,
                )
            for kc in range(KC):
                nc.tensor.matmul(
                    out=p_out[:],
                    lhsT=featT_t[:, kc * P : (kc + 1) * P],
                    rhs=wr_sb[:, kc * dim : (kc + 1) * dim],
                    start=False,
                    stop=False,
                )
            nc.tensor.matmul(
                out=p_out[:],
                lhsT=ones_row[:],
                rhs=bias_sb[:],
                start=False,
                stop=True,
            )
            h_sb = work.tile([P, dim], F32, tag="hout")
            nc.vector.tensor_scalar(
                out=h_sb[:],
                in0=p_out[:],
                scalar1=0.5,
                scalar2=None,
                op0=mybir.AluOpType.mult,
            )
            nc.vector.tensor_tensor(
                out=h_sb[:],
                in0=h_sb[:],
                in1=p_out[:],
                op=mybir.AluOpType.max,
            )
            nc.sync.dma_start(out=hout[t * P : (t + 1) * P, :], in_=h_sb[:])

        col = 0
        for j in range(npairs):
            a, b = 2 * j, 2 * j + 1
            Kp = K_pair[j]
            m_tile = work.tile([P, 2 * Kp_max * dim], BF16, tag="gather", bufs=3)
            nc.sync.dma_start(
                out=m_tile[:, : 2 * Kp * dim],
                in_=msg[:, col * dim : (col + 2 * Kp) * dim],
            )
            p_agg = psum.tile([P, 2 * dim], F32, tag="agg")
            for k in range(Kp):
                nc.tensor.matmul(
                    out=p_agg[:],
                    lhsT=ident[:],
                    rhs=m_tile[:, 2 * k * dim : 2 * (k + 1) * dim],
                    start=(k == 0),
                    stop=(k == Kp - 1),
                )
            mean_sb = work.tile([P, 2 * dim], F32, tag="mean")
            nc.vector.tensor_scalar(
                out=mean_sb[:, :dim],
                in0=p_agg[:, :dim],
                scalar1=recip_sb[:, a : a + 1],
                scalar2=None,
                op0=mybir.AluOpType.mult,
            )
            nc.vector.tensor_scalar(
                out=mean_sb[:, dim:],
                in0=p_agg[:, dim:],
                scalar1=recip_sb[:, b : b + 1],
                scalar2=None,
                op0=mybir.AluOpType.mult,
            )
            tile_tail(a, mean_sb[:, :dim])
            tile_tail(b, mean_sb[:, dim:])
            col += 2 * Kp
        if T % 2:
            t = T - 1
            Kt = K_list[t]
            m_tile = work.tile([P, 2 * Kp_max * dim], BF16, tag="gather", bufs=3)
            nc.sync.dma_start(
                out=m_tile[:, : Kt * dim],
                in_=msg[:, col * dim : (col + Kt) * dim],
            )
            p_agg1 = psum.tile([P, dim], F32, tag="agg1")
            for k in range(Kt):
                nc.tensor.matmul(
                    out=p_agg1[:],
                    lhsT=ident[:],
                    rhs=m_tile[:, k * dim : (k + 1) * dim],
                    start=(k == 0),
                    stop=(k == Kt - 1),
                )
            mean_sb = work.tile([P, 2 * dim], F32, tag="mean")
            nc.vector.tensor_scalar(
                out=mean_sb[:, :dim],
                in0=p_agg1[:],
                scalar1=recip_sb[:, t : t + 1],
                scalar2=None,
                op0=mybir.AluOpType.mult,
            )
            tile_tail(t, mean_sb[:, :dim])
    nc.finalize()
    return nc


# ----------------------------------------------------------------- execution
def _layer_inputs(meta, feat_full, wl, wr, bl, n_nodes):
    """Build per-core in_maps for one layer launch (host does the gather)."""
    feat_aug = np.zeros((n_nodes + 1, feat_full.shape[1]), dtype=BF)
    feat_aug[:n_nodes] = feat_full.astype(BF)
    wl16 = np.ascontiguousarray(wl, dtype=BF)
    wr16 = np.ascontiguousarray(wr, dtype=BF)
    bl16 = np.asarray(bl, dtype=BF).reshape(1, -1)
    in_maps = []
    for c in range(len(meta["slots"])):
        nodes = meta["node_order"][c]
        shard = feat_full[np.maximum(nodes, 0)]
        shard[nodes < 0] = 0.0
        msg = feat_aug[meta["slots"][c]]  # [P, C_total, dim] bf16
        in_maps.append(
            dict(
                msg=np.ascontiguousarray(msg.reshape(P, -1)),
                featT=np.ascontiguousarray(shard.T.astype(BF)),
                recip=meta["recip"][c],
                wl=wl16,
                wr=wr16,
                bl=bl16,
            )
        )
    return in_maps


def _unshard(meta, results, n_nodes, dim):
    h = np.zeros((n_nodes, dim), dtype=np.float32)
    for c, r in enumerate(results):
        nodes = meta["node_order"][c]
        valid = nodes >= 0
        h[nodes[valid]] = r["hout"][valid]
    return h


def _run_layers(x, edge_index, layer_params, n_nodes, dim, n_cores, run_kwargs=None):
    meta = _prep_graph(edge_index, n_nodes, n_cores)
    nc = build_layer_nc(meta["K_list"], meta["nsh_pad"], dim, n_cores)
    h = np.asarray(x, dtype=np.float32)
    core_ids = list(range(n_cores))
    extra = []
    for wl, bl, wr in layer_params:
        in_maps = _layer_inputs(meta, h, wl, wr, bl, n_nodes)
        res = None
        for attempt in range(3):
            try:
                res = run_bass_kernel_spmd(nc, in_maps, core_ids, **(run_kwargs or {}))
                break
            except Exception:
                if attempt == 2:
                    raise
                # a wedged accelerator recovers on a fresh PJRT client; force
                # a backend re-init before retrying
                import time as _time

                _time.sleep(5)
                try:
                    import jax as _jax
                    from jax._src import xla_bridge as _xb

                    _jax.clear_caches()
                    _xb._clear_backends()
                except Exception:
                    pass
        h = _unshard(meta, res.results, n_nodes, dim)
        extra.append(res)
    return h, extra


def kernel(x, edge_index, Wl0, bl0, Wr0, Wl1, bl1, Wr1, _run_kwargs=None, _extra=None):
    x = np.asarray(x, dtype=np.float32)
    h, extra = _run_layers(
        x,
        np.asarray(edge_index),
        [(Wl0, bl0, Wr0), (Wl1, bl1, Wr1)],
        N_NODES,
        DIM,
        N_CORES,
        run_kwargs=_run_kwargs,
    )
    if _extra is not None:
        _extra.extend(extra)
    return h, x


# revision 26
# speedup vs baseline: 1.1144x; 1.1144x over previous
"""GraphSAGE 2-layer encoder on 8 Trainium2 NeuronCores.

Reference computation (PyG SAGEConv, aggr='mean', 2 layers, leaky-relu 0.5):
    h = x
    for layer in (0, 1):
        mean_i = (1/max(deg_i,1)) * sum_{j in N(i)} h_j
        h = leaky( mean @ Wl + h @ Wr + bl )
    return (h, x)

Strategy: shard the 50000 dst nodes across 8 cores (6250 each). Host sorts
each core's nodes by in-degree (round-robin by global degree rank, so every
core's tile t covers the same degree band) and assigns every edge a
(tile, slot, partition) so a message tile [128, Kt*256] is node-aligned:
slot (p, k) holds the src features of node p's k-th in-edge.

On-device random gathers bottleneck on Q7 software descriptor generation
(~9.3 ns per 512B row descriptor -> ~1 ms/layer/core for 100K edges), so the
host performs the slot gather between launches (allowed by the full-inputs
contract -- the host already re-shards h between the two launches) and the
device streams the pre-gathered message array with large affine DMAs.
Per tile: one contiguous DMA, a PSUM-accumulating identity-matmul chain for
the segment sum, deg-reciprocal scale, PE transpose, and the two GEMMs +
bias + leaky-relu. Features/weights are bf16 (1 PE cycle/row vs 4 for
fp32); accumulation stays fp32 in PSUM.

Each layer is one SPMD bass launch; the h exchange between layers goes
through the host.
"""

import numpy as np
from contextlib import ExitStack

import ml_dtypes

import concourse.bass as bass
import concourse.bacc as bacc
import concourse.mybir as mybir
import concourse.tile as tile
from concourse.bass_utils import run_bass_kernel_spmd
from concourse.masks import make_identity

P = 128
N_NODES = 50000
DIM = 256
N_CORES = 8

F32 = mybir.dt.float32
BF16 = mybir.dt.bfloat16
BF = ml_dtypes.bfloat16


# ---------------------------------------------------------------- host prep
def _prep_graph(edge_index, n_nodes, n_cores):
    """Slot assignment: returns per-core slot grid [P, C_total] of global
    node ids (pad -> n_nodes, the zero row), recip [P, T], node_order,
    K_list (chunk count per tile, shared by all cores)."""
    src = np.asarray(edge_index[0], dtype=np.int64)
    dst = np.asarray(edge_index[1], dtype=np.int64)
    deg = np.bincount(dst, minlength=n_nodes)

    order = np.argsort(dst, kind="stable")
    srcs_sorted = src[order].astype(np.int64)
    cum = np.zeros(n_nodes + 1, dtype=np.int64)
    np.cumsum(deg, out=cum[1:])

    nsh = n_nodes // n_cores
    T = (nsh + P - 1) // P
    nsh_pad = T * P

    # node -> core by global degree rank, round-robin: tile t then holds the
    # same degree band on every core, so the shared per-tile chunk count
    # K_t = max-degree-in-tile has no cross-core slack
    node_order = np.full((n_cores, nsh_pad), -1, dtype=np.int64)
    deg_slot = np.zeros((n_cores, nsh_pad), dtype=np.int64)
    rank = np.argsort(-deg, kind="stable")
    for c in range(n_cores):
        g = rank[c::n_cores][:nsh]
        node_order[c, :nsh] = g
        deg_slot[c, :nsh] = deg[g]

    K_list = []
    for t in range(T):
        K_t = int(deg_slot[:, t * P : (t + 1) * P].max())
        K_list.append(max(K_t, 1))
    C_total = int(np.sum(K_list))
    col_off = np.concatenate([[0], np.cumsum(K_list)]).astype(np.int64)

    slots = np.full((n_cores, P, C_total), n_nodes, dtype=np.int64)
    recip_arr = np.zeros((n_cores, P, T), dtype=np.float32)
    for c in range(n_cores):
        for t in range(T):
            Kt = K_list[t]
            nodes = node_order[c, t * P : (t + 1) * P]
            degs = deg_slot[c, t * P : (t + 1) * P]
            recip_arr[c, :, t] = 1.0 / np.maximum(degs, 1)
            for p in range(P):
                nd = nodes[p]
                if nd < 0:
                    continue
                d = int(degs[p])
                if d:
                    slots[c, p, col_off[t] : col_off[t] + d] = srcs_sorted[
                        cum[nd] : cum[nd] + d
                    ]

    return dict(
        slots=slots,
        recip=recip_arr,
        node_order=node_order,
        K_list=K_list,
        T=T,
        nsh=nsh,
        nsh_pad=nsh_pad,
        C_total=C_total,
    )


# ------------------------------------------------------------ device program
def build_layer_nc(K_list, nsh_pad, dim=DIM, n_cores=N_CORES, t_limit=None):
    """One SAGEConv layer (mean-aggregate + linear + leaky 0.5) over a
    host-pre-gathered slot-aligned message array."""
    T = len(K_list)
    if t_limit is not None:
        T = min(T, t_limit)
        K_list = K_list[:T]
    C_total = int(np.sum(K_list))
    K_max = int(np.max(K_list))
    assert dim % P == 0
    KC = dim // P

    nc = bacc.Bacc(
        "TRN2",
        target_bir_lowering=False,
        debug=False,
        enable_asserts=False,
        num_devices=n_cores,
    )
    msg = nc.dram_tensor("msg", [P, C_total * dim], BF16, kind="ExternalInput").ap()
    featT = nc.dram_tensor("featT", [dim, nsh_pad], BF16, kind="ExternalInput").ap()
    recip = nc.dram_tensor("recip", [P, T], F32, kind="ExternalInput").ap()
    wl = nc.dram_tensor("wl", [dim, dim], BF16, kind="ExternalInput").ap()
    wr = nc.dram_tensor("wr", [dim, dim], BF16, kind="ExternalInput").ap()
    bl = nc.dram_tensor("bl", [1, dim], BF16, kind="ExternalInput").ap()
    hout = nc.dram_tensor("hout", [nsh_pad, dim], F32, kind="ExternalOutput").ap()

    with tile.TileContext(nc) as tc, ExitStack() as ctx:
        const = ctx.enter_context(tc.tile_pool(name="const", bufs=1))
        work = ctx.enter_context(tc.tile_pool(name="work", bufs=3))
        psum = ctx.enter_context(tc.tile_pool(name="psum", bufs=2, space="PSUM"))

        ident = const.tile([P, P], BF16)
        make_identity(nc, ident[:])
        ident32 = const.tile([P, P], F32)
        make_identity(nc, ident32[:])
        ones_row = const.tile([1, P], BF16)
        nc.gpsimd.memset(ones_row[:], 1.0)

        recip_sb = const.tile([P, T], F32)
        nc.sync.dma_start(out=recip_sb[:], in_=recip[:, :])
        bias_sb = const.tile([1, dim], BF16)
        nc.sync.dma_start(out=bias_sb[:], in_=bl[:, :])

        wl_sb = const.tile([P, KC * dim], BF16)
        wr_sb = const.tile([P, KC * dim], BF16)
        for kc in range(KC):
            nc.sync.dma_start(
                out=wl_sb[:, kc * dim : (kc + 1) * dim],
                in_=wl[kc * P : (kc + 1) * P, :],
            )
            nc.sync.dma_start(
                out=wr_sb[:, kc * dim : (kc + 1) * dim],
                in_=wr[kc * P : (kc + 1) * P, :],
            )

        col = 0
        for t in range(T):
            Kt = K_list[t]
            # stream this tile's pre-gathered messages: one affine DMA
            m_tile = work.tile([P, K_max * dim], BF16, tag="gather", bufs=4)
            nc.sync.dma_start(
                out=m_tile[:, : Kt * dim],
                in_=msg[:, col * dim : (col + Kt) * dim],
            )
            # this tile's xT block for the Wr term (streamed, not resident)
            featT_t = work.tile([P, KC * P], BF16, tag="featT")
            for kc in range(KC):
                nc.sync.dma_start(
                    out=featT_t[:, kc * P : (kc + 1) * P],
                    in_=featT[kc * P : (kc + 1) * P, t * P : (t + 1) * P],
                )
            # segment-sum: accumulate chunks into PSUM with identity lhsT
            p_agg = psum.tile([P, dim], F32, tag="agg")
            for k in range(Kt):
                nc.tensor.matmul(
                    out=p_agg[:],
                    lhsT=ident[:],
                    rhs=m_tile[:, k * dim : (k + 1) * dim],
                    start=(k == 0),
                    stop=(k == Kt - 1),
                )
            # mean = agg * (1/deg)
            mean_sb = work.tile([P, dim], F32, tag="mean")
            nc.vector.tensor_scalar(
                out=mean_sb[:],
                in0=p_agg[:],
                scalar1=recip_sb[:, t : t + 1],
                scalar2=None,
                op0=mybir.AluOpType.mult,
            )
            # meanT via PE transpose (two 128x128 blocks), cast bf16 on copy
            meanT_sb = work.tile([P, KC * P], BF16, tag="meanT")
            for kc in range(KC):
                p_tr = psum.tile([P, P], F32, tag="tr")
                nc.tensor.transpose(
                    out=p_tr[:],
                    in_=mean_sb[:, kc * P : (kc + 1) * P],
                    identity=ident32[:],
                )
                nc.vector.tensor_copy(
                    out=meanT_sb[:, kc * P : (kc + 1) * P], in_=p_tr[:]
                )
            # out = mean @ Wl + x @ Wr + b
            p_out = psum.tile([P, dim], F32, tag="out")
            for kc in range(KC):
                nc.tensor.matmul(
                    out=p_out[:],
                    lhsT=meanT_sb[:, kc * P : (kc + 1) * P],
                    rhs=wl_sb[:, kc * dim : (kc + 1) * dim],
                    start=(kc == 0),
                    stop=False,
                )
            for kc in range(KC):
                nc.tensor.matmul(
                    out=p_out[:],
                    lhsT=featT_t[:, kc * P : (kc + 1) * P],
                    rhs=wr_sb[:, kc * dim : (kc + 1) * dim],
                    start=False,
                    stop=False,
                )
            nc.tensor.matmul(
                out=p_out[:],
                lhsT=ones_row[:],
                rhs=bias_sb[:],
                start=False,
                stop=True,
            )
            # leaky relu slope 0.5: max(0.5*h, h)
            h_sb = work.tile([P, dim], F32, tag="hout")
            nc.vector.tensor_scalar(
                out=h_sb[:],
                in0=p_out[:],
                scalar1=0.5,
                scalar2=None,
                op0=mybir.AluOpType.mult,
            )
            nc.vector.tensor_tensor(
                out=h_sb[:],
                in0=h_sb[:],
                in1=p_out[:],
                op=mybir.AluOpType.max,
            )
            nc.sync.dma_start(out=hout[t * P : (t + 1) * P, :], in_=h_sb[:])
            col += Kt
    nc.finalize()
    return nc


# ----------------------------------------------------------------- execution
def _layer_inputs(meta, feat_full, wl, wr, bl, n_nodes):
    """Build per-core in_maps for one layer launch (host does the gather)."""
    feat_aug = np.zeros((n_nodes + 1, feat_full.shape[1]), dtype=BF)
    feat_aug[:n_nodes] = feat_full.astype(BF)
    wl16 = np.ascontiguousarray(wl, dtype=BF)
    wr16 = np.ascontiguousarray(wr, dtype=BF)
    bl16 = np.asarray(bl, dtype=BF).reshape(1, -1)
    in_maps = []
    for c in range(len(meta["slots"])):
        nodes = meta["node_order"][c]
        shard = feat_full[np.maximum(nodes, 0)]
        shard[nodes < 0] = 0.0
        msg = feat_aug[meta["slots"][c]]  # [P, C_total, dim] bf16
        in_maps.append(
            dict(
                msg=np.ascontiguousarray(msg.reshape(P, -1)),
                featT=np.ascontiguousarray(shard.T.astype(BF)),
                recip=meta["recip"][c],
                wl=wl16,
                wr=wr16,
                bl=bl16,
            )
        )
    return in_maps


def _unshard(meta, results, n_nodes, dim):
    h = np.zeros((n_nodes, dim), dtype=np.float32)
    for c, r in enumerate(results):
        nodes = meta["node_order"][c]
        valid = nodes >= 0
        h[nodes[valid]] = r["hout"][valid]
    return h


def _run_layers(x, edge_index, layer_params, n_nodes, dim, n_cores, run_kwargs=None):
    meta = _prep_graph(edge_index, n_nodes, n_cores)
    nc = build_layer_nc(meta["K_list"], meta["nsh_pad"], dim, n_cores)
    h = np.asarray(x, dtype=np.float32)
    core_ids = list(range(n_cores))
    extra = []
    for wl, bl, wr in layer_params:
        in_maps = _layer_inputs(meta, h, wl, wr, bl, n_nodes)
        res = None
        for attempt in range(3):
            try:
                res = run_bass_kernel_spmd(nc, in_maps, core_ids, **(run_kwargs or {}))
                break
            except Exception:
                if attempt == 2:
                    raise
                # a wedged accelerator recovers on a fresh PJRT client; force
                # a backend re-init before retrying
                import time as _time

                _time.sleep(5)
                try:
                    import jax as _jax
                    from jax._src import xla_bridge as _xb

                    _jax.clear_caches()
                    _xb._clear_backends()
                except Exception:
                    pass
        h = _unshard(meta, res.results, n_nodes, dim)
        extra.append(res)
    return h, extra


def kernel(x, edge_index, Wl0, bl0, Wr0, Wl1, bl1, Wr1, _run_kwargs=None, _extra=None):
    x = np.asarray(x, dtype=np.float32)
    h, extra = _run_layers(
        x,
        np.asarray(edge_index),
        [(Wl0, bl0, Wr0), (Wl1, bl1, Wr1)],
        N_NODES,
        DIM,
        N_CORES,
        run_kwargs=_run_kwargs,
    )
    if _extra is not None:
        _extra.extend(extra)
    return h, x
